# revision 1
# baseline (speedup 1.0000x reference)
"""Trainium2 Bass kernel for a 2-block decoder layer (masked cross-attn +
cross-attn + FFN), data-parallel over batch: 8 batch elements on 8 NeuronCores.

Shapes (hardcoded): B=8, S=1024, D=1024, H=16 heads, DH=64, DFF=2048.

Per-core plan (PE matmuls in float32r at full 1 cyc/row rate):
  - inputs pre-transposed on host: xT [D, S]; all large inputs declared
    float32r so plain HWDGE DMAs feed the PE directly (HW rounds on use)
  - qT/kT computed as [H*DH, S] (head pair per 128-partition tile)
  - scoresT: two k-chunks packed per [128,1024] PSUM tile via row-packed
    K=64 matmul pairs (tile_position (0,0)/(64,0)); one exp per tile
  - causal: fully-masked chunks skipped, diagonal chunks multiplied by 0/1
    mask patterns (two combined [128,1024] patterns)
  - attn@V + softmax denominator fused: per pair, v stored [v_h|ones|v_h'],
    one M=128 matmul per head yields out'T on one 64-partition half and the
    denominator (replicated) on the other
  - FFN: zT = relu(W1^T @ h2T + b1) on ScalarE, yT = W2^T @ zrT + b2
  - output yT [D, S] -> host transposes back
"""

from contextlib import ExitStack

import numpy as np

B, S, D, H, DH, DFF = 8, 1024, 1024, 16, 64, 2048
NP = 128
NKC = D // NP       # 8 chunks along D
NFC = DFF // NP     # 16 chunks along DFF
NQC = 2             # free-dim chunks of 512
QB = S // NQC       # 512
NPAIR = H // 2      # 8 head pairs
VPW = 192           # [v_h | ones64 | v_h'] per-pair width

_CACHE = {}
_PHASES = 3


def _build():
    import concourse.bacc as bacc
    import concourse.bass as bass
    import concourse.mybir as mybir
    import concourse.tile as tile

    f32 = mybir.dt.float32
    f32r = mybir.dt.float32r
    AF = mybir.ActivationFunctionType

    nc = bacc.Bacc("TRN2", num_swdge_queues=4)

    xdeT_d = nc.dram_tensor("xdeT", [D, S], f32r, kind="ExternalInput")
    xenT_d = nc.dram_tensor("xenT", [D, S], f32r, kind="ExternalInput")
    w_d = {}
    for nm in ("wq1", "wk1", "wv1", "wq2", "wk2", "wv2"):
        w_d[nm] = nc.dram_tensor(nm, [D, H * DH], f32r, kind="ExternalInput")
    w1_d = nc.dram_tensor("w1", [D, DFF], f32r, kind="ExternalInput")
    w2_d = nc.dram_tensor("w2", [DFF, D], f32r, kind="ExternalInput")
    bq1_d = nc.dram_tensor("bq1", [NP, NKC], f32, kind="ExternalInput")
    bk1_d = nc.dram_tensor("bk1", [NP, NKC], f32, kind="ExternalInput")
    bv1_d = nc.dram_tensor("bv1", [1, H * DH], f32r, kind="ExternalInput")
    bq2_d = nc.dram_tensor("bq2", [NP, NKC], f32, kind="ExternalInput")
    bk2_d = nc.dram_tensor("bk2", [NP, NKC], f32, kind="ExternalInput")
    bv2_d = nc.dram_tensor("bv2", [1, H * DH], f32r, kind="ExternalInput")
    b1_d = nc.dram_tensor("b1", [NP, NFC], f32, kind="ExternalInput")
    b2_d = nc.dram_tensor("b2", [NP, NKC], f32, kind="ExternalInput")
    m01_d = nc.dram_tensor("m01", [2, NP, S], f32r, kind="ExternalInput")
    yT_d = nc.dram_tensor("yT", [D, S], f32, kind="ExternalOutput")
    h1_d = nc.dram_tensor("h1s", [D, S], f32r, kind="Internal")
    h2_d = nc.dram_tensor("h2s", [D, S], f32r, kind="Internal")

    with tile.TileContext(nc, pool_alloc_mode="queue") as tc, ExitStack() as X:
        P = X.enter_context(tc.tile_pool(name="persist", bufs=1))
        EX = X.enter_context(tc.tile_pool(name="expp", bufs=5))
        SM = X.enter_context(tc.tile_pool(name="small", bufs=2))
        PSUM = X.enter_context(tc.tile_pool(name="psum", bufs=1, space="PSUM"))

        def psum_tile(tag, bufs, width=QB):
            return PSUM.tile([NP, width], f32, name=tag, tag=tag, bufs=bufs)

        # ---------- constants / biases ----------
        ones_f = P.tile([NP, 64], f32, name="ones_f")
        nc.vector.memset(ones_f, 1.0)
        onecol_f = P.tile([1, NP], f32, name="onecol_f")
        nc.vector.memset(onecol_f, 1.0)
        onecol = P.tile([1, NP], f32r, name="onecol")
        nc.gpsimd.tensor_copy(onecol, onecol_f)

        masks = []
        for i in range(2):
            mr = P.tile([NP, S], f32r, name=f"m01_{i}")
            nc.sync.dma_start(out=mr, in_=m01_d[i, :, :])
            masks.append(mr)

        def load_bias(d, shape, name):
            t = P.tile(list(shape), f32, name=name)
            nc.sync.dma_start(out=t, in_=d[:, :])
            return t

        bq1 = load_bias(bq1_d, (NP, NKC), "bq1")
        bk1 = load_bias(bk1_d, (NP, NKC), "bk1")
        bq2 = load_bias(bq2_d, (NP, NKC), "bq2")
        bk2 = load_bias(bk2_d, (NP, NKC), "bk2")
        b1 = load_bias(b1_d, (NP, NFC), "b1")
        b2 = load_bias(b2_d, (NP, NKC), "b2")

        def load_bv(d, name):
            br = P.tile([1, H * DH], f32r, name=name)
            nc.sync.dma_start(out=br, in_=d[:, :])
            return br

        bv1 = load_bv(bv1_d, "bv1")
        bv2 = load_bv(bv2_d, "bv2")

        # ---------- helpers ----------
        def dma_load(out, in_):
            nc.sync.dma_start(out=out, in_=in_)

        def load_cast(src_slice, pool, name, width, tag=None):
            xr = pool.tile([NP, width], f32r, name=name, tag=tag or name)
            dma_load(xr, src_slice)
            return xr

        def load_h(src, pool, name):
            tiles = []
            for kc in range(NKC):
                t = pool.tile([NP, S], f32r, name=f"{name}{kc}", tag=f"{name}{kc}")
                dma_load(t, src[kc * NP:(kc + 1) * NP, :])
                tiles.append(t)
            return tiles

        def load_xT(src, pool, name):
            tiles = []
            for kc in range(NKC):
                t = pool.tile([NP, S], f32r, name=f"{name}{kc}", tag=f"{name}{kc}")
                dma_load(t, src[kc * NP:(kc + 1) * NP, :])
                tiles.append(t)
            return tiles

        def proj_T(wd, rhsT, bias, outname, outpool, wpool):
            """out[mc] [128, S] f32r = W^T @ rhsT + bias(per partition)."""
            wch = [
                load_cast(wd[kc * NP:(kc + 1) * NP, :], wpool, f"{outname}w{kc}",
                          H * DH)
                for kc in range(NKC)
            ]
            outs = []
            for mc in range(NKC):
                ot = outpool.tile([NP, S], f32r, name=f"{outname}{mc}")
                ps = psum_tile("mm", 2, S)
                for qc in range(NQC):
                    for kc in range(NKC):
                        nc.tensor.matmul(
                            ps[:, qc * QB:(qc + 1) * QB],
                            wch[kc][:, mc * NP:(mc + 1) * NP],
                            rhsT[kc][:, qc * QB:(qc + 1) * QB],
                            start=(kc == 0),
                            stop=(kc == NKC - 1),
                        )
                nc.vector.tensor_scalar_add(ot, ps, bias[:, mc:mc + 1])
                outs.append(ot)
            return outs

        def proj_v(wd, lhsT, bvr, outname, outpool, wpool):
            """vp[mc] [128, NPAIR*VPW] f32r: [v_h|ones|v_h'] per pair.
            Weights streamed per column half (512 H*DH columns)."""
            vps = []
            for mc in range(NKC):
                vp = outpool.tile([NP, NPAIR * VPW], f32r, name=f"{outname}{mc}")
                for p in range(NPAIR):
                    nc.gpsimd.tensor_copy(vp[:, p * VPW + 64:p * VPW + 128],
                                          ones_f)
                vps.append(vp)
            for qc in range(NQC):
                wch = [
                    load_cast(
                        wd[kc * NP:(kc + 1) * NP, qc * QB:(qc + 1) * QB],
                        wpool, f"{outname}w{qc}_{kc}", QB, tag=f"{outname}wh{kc}",
                    )
                    for kc in range(NKC)
                ]
                for mc in range(NKC):
                    ps = psum_tile("pv", 4)
                    for kc in range(NKC):
                        nc.tensor.matmul(
                            ps,
                            lhsT[kc][:, mc * NP:(mc + 1) * NP],
                            wch[kc],
                            start=(kc == 0),
                            stop=False,
                        )
                    nc.tensor.matmul(
                        ps, onecol, bvr[:, qc * QB:(qc + 1) * QB],
                        start=False, stop=True,
                    )
                    # single strided copy: psum [4 pairs][2 halves][64] ->
                    # vp columns {192p, 192p+128} + 64
                    src = bass.AP(
                        tensor=ps.tensor, offset=ps.offset,
                        ap=[list(ps.ap[0]), [NP, 4], [64, 2], [1, 64]],
                    )
                    vpt = vps[mc]
                    dst = bass.AP(
                        tensor=vpt.tensor,
                        offset=vpt.offset + 4 * qc * VPW,
                        ap=[list(vpt.ap[0]), [VPW, 4], [128, 2], [1, 64]],
                    )
                    nc.vector.tensor_copy(dst, src)
            return vps

        def attention(qT, kT, vp, masked, outname, out_dram):
            for p in range(NPAIR):
                hs = SM.tile([NP, S], f32r, name="hstg", tag="hstg", bufs=1)
                for qc in range(NQC):
                    kcs = [0, 1, 2, 3] if (masked and qc == 0) else list(range(NKC))
                    kpairs = [(kcs[2 * j], kcs[2 * j + 1])
                              for j in range(len(kcs) // 2)]
                    pv0 = psum_tile("pv", 4)
                    pv1 = psum_tile("pv", 4)
                    es = {}

                    def scores(j):
                        ka, kb = kpairs[j]
                        ps0 = psum_tile("mm", 2, S)
                        ps1 = psum_tile("mm", 2, S)
                        for half, kc in ((0, ka), (1, kb)):
                            nc.tensor.matmul(
                                ps0[:, half * QB:(half + 1) * QB],
                                kT[p][0:64, kc * NP:(kc + 1) * NP],
                                qT[p][0:64, qc * QB:(qc + 1) * QB],
                                start=True, stop=True, tile_position=(0, 0),
                            )
                            nc.tensor.matmul(
                                ps1[:, half * QB:(half + 1) * QB],
                                kT[p][64:128, kc * NP:(kc + 1) * NP],
                                qT[p][64:128, qc * QB:(qc + 1) * QB],
                                start=True, stop=True, tile_position=(64, 0),
                            )
                        e0 = EX.tile([NP, S], f32r, name="e0", tag="ex")
                        e1 = EX.tile([NP, S], f32r, name="e1", tag="ex")
                        nc.scalar.activation(e0, ps0, AF.Exp, scale=0.125)
                        nc.scalar.activation(e1, ps1, AF.Exp, scale=0.125)
                        if masked:
                            o = ka * NP - qc * QB
                            if 0 <= o < QB:
                                m = masks[o // 256]
                                nc.vector.tensor_mul(e0, e0, m)
                                nc.vector.tensor_mul(e1, e1, m)
                        es[j] = (e0, e1)

                    scores(0)
                    nj = len(kpairs)
                    for j in range(nj):
                        if j + 1 < nj:
                            scores(j + 1)
                        e0, e1 = es.pop(j)
                        ka, kb = kpairs[j]
                        for half, kc in ((0, ka), (1, kb)):
                            last = (j == nj - 1) and (half == 1)
                            nc.tensor.matmul(
                                pv0, vp[kc][:, p * VPW:p * VPW + 128],
                                e0[:, half * QB:(half + 1) * QB],
                                start=(j == 0 and half == 0), stop=last,
                            )
                            nc.tensor.matmul(
                                pv1, vp[kc][:, p * VPW + 64:p * VPW + 192],
                                e1[:, half * QB:(half + 1) * QB],
                                start=(j == 0 and half == 0), stop=last,
                            )
                    rec0 = SM.tile([64, QB], f32, name="rec0", tag="rec", bufs=2)
                    rec1 = SM.tile([64, QB], f32, name="rec1", tag="rec", bufs=2)
                    nc.vector.reciprocal(rec0, pv0[64:128, :])
                    nc.vector.reciprocal(rec1, pv1[0:64, :])
                    nc.vector.tensor_mul(
                        hs[0:64, qc * QB:(qc + 1) * QB], pv0[0:64, :], rec0
                    )
                    nc.vector.tensor_mul(
                        hs[64:128, qc * QB:(qc + 1) * QB],
                        pv1[64:128, :], rec1,
                    )
                nc.sync.dma_start(
                    out=out_dram[p * NP:(p + 1) * NP, :], in_=hs
                )

        # ================= block 1 =================
        XENpool = tc.tile_pool(name="xenp", bufs=1)
        XEN = XENpool.__enter__()
        B1pool = tc.tile_pool(name="b1qk", bufs=1)
        B1 = B1pool.__enter__()
        with tc.tile_pool(name="xdep", bufs=1) as XD, \
             tc.tile_pool(name="wq1p", bufs=1) as WQ1:
            xdeT = load_xT(xdeT_d, XD, "xdeT")
            q1T = proj_T(w_d["wq1"], xdeT, bq1, "q1T", B1, WQ1)
        xenT = load_xT(xenT_d, XEN, "xenT")
        with tc.tile_pool(name="wk1p", bufs=1) as WK1:
            k1T = proj_T(w_d["wk1"], xenT, bk1, "k1T", B1, WK1)
        VP1pool = tc.tile_pool(name="vp1p", bufs=1)
        VP1 = VP1pool.__enter__()
        with tc.tile_pool(name="wv1p", bufs=1) as WV1:
            vp1 = proj_v(w_d["wv1"], xenT, bv1, "vp1", VP1, WV1)
        attention(q1T, k1T, vp1, True, "h1T", h1_d)
        VP1pool.__exit__(None, None, None)
        B1pool.__exit__(None, None, None)

        if _PHASES < 2:
            XENpool.__exit__(None, None, None)
        if _PHASES >= 2:
            # ================= block 2 =================
            B2pool = tc.tile_pool(name="b2qk", bufs=1)
            B2 = B2pool.__enter__()
            with tc.tile_pool(name="h1lp", bufs=1) as HL1, \
                 tc.tile_pool(name="wq2p", bufs=1) as WQ2:
                h1T = load_h(h1_d, HL1, "h1l")
                q2T = proj_T(w_d["wq2"], h1T, bq2, "q2T", B2, WQ2)
            with tc.tile_pool(name="wk2p", bufs=1) as WK2:
                k2T = proj_T(w_d["wk2"], xenT, bk2, "k2T", B2, WK2)
            VP2pool = tc.tile_pool(name="vp2p", bufs=1)
            VP2 = VP2pool.__enter__()
            with tc.tile_pool(name="wv2p", bufs=1) as WV2:
                vp2 = proj_v(w_d["wv2"], xenT, bv2, "vp2", VP2, WV2)
            attention(q2T, k2T, vp2, False, "h2T", h2_d)
            VP2pool.__exit__(None, None, None)
            B2pool.__exit__(None, None, None)
            XENpool.__exit__(None, None, None)

        if _PHASES >= 3:
            # ================= FFN =================
            with tc.tile_pool(name="zrp", bufs=1) as ZR:
                zrT = [None] * NFC
                with tc.tile_pool(name="h2lp", bufs=1) as HL2:
                    h2T = load_h(h2_d, HL2, "h2l")
                    for half in range(2):
                        with tc.tile_pool(name=f"w1p{half}", bufs=1) as W1P:
                            w1ch = [
                                load_cast(
                                    w1_d[kc * NP:(kc + 1) * NP,
                                         half * 1024:(half + 1) * 1024],
                                    W1P, f"w1c{half}_{kc}", 1024,
                                )
                                for kc in range(NKC)
                            ]
                            for ml in range(8):
                                mc = half * 8 + ml
                                zt = ZR.tile([NP, S], f32r, name=f"zrT{mc}")
                                ps = psum_tile("mm", 2, S)
                                for qc in range(NQC):
                                    for kc in range(NKC):
                                        nc.tensor.matmul(
                                            ps[:, qc * QB:(qc + 1) * QB],
                                            w1ch[kc][:, ml * NP:(ml + 1) * NP],
                                            h2T[kc][:, qc * QB:(qc + 1) * QB],
                                            start=(kc == 0),
                                            stop=(kc == NKC - 1),
                                        )
                                nc.scalar.activation(
                                    zt, ps, AF.Relu,
                                    bias=b1[:, mc:mc + 1], scale=1.0,
                                )
                                zrT[mc] = zt

                for half in range(2):
                    with tc.tile_pool(name=f"w2p{half}", bufs=1) as W2P:
                        w2ch = [
                            load_cast(
                                w2_d[kc * NP:(kc + 1) * NP,
                                     half * 512:(half + 1) * 512],
                                W2P, f"w2c{half}_{kc}", 512,
                            )
                            for kc in range(NFC)
                        ]
                        for ml in range(4):
                            mc = half * 4 + ml
                            yt = W2P.tile([NP, S], f32, name="yt", tag="yt",
                                          bufs=2)
                            ps = psum_tile("mm", 2, S)
                            for qc in range(NQC):
                                for kc in range(NFC):
                                    nc.tensor.matmul(
                                        ps[:, qc * QB:(qc + 1) * QB],
                                        w2ch[kc][:, ml * NP:(ml + 1) * NP],
                                        zrT[kc][:, qc * QB:(qc + 1) * QB],
                                        start=(kc == 0),
                                        stop=(kc == NFC - 1),
                                    )
                            nc.vector.tensor_scalar_add(
                                yt, ps, b2[:, mc:mc + 1]
                            )
                            nc.sync.dma_start(
                                out=yT_d[mc * NP:(mc + 1) * NP, :], in_=yt
                            )

    nc.compile()
    return nc


def _prep_inputs(de_x, en_x, mask, attn1_w, attn1_b, attn2_w, attn2_b,
                 ffn_w1, ffn_b1, ffn_w2, ffn_b2):
    f = np.float32

    def wflat(w):  # [H, D, DH] -> [D, H*DH]
        return np.ascontiguousarray(
            np.asarray(w).transpose(1, 0, 2).reshape(D, H * DH), dtype=f
        )

    def bpart(b):  # [H, DH] -> [128, 8] partition-major
        return np.ascontiguousarray(
            np.asarray(b).reshape(H * DH).reshape(NKC, NP).T, dtype=f
        )

    shared = {
        "wq1": wflat(attn1_w[0]), "wk1": wflat(attn1_w[1]),
        "wv1": wflat(attn1_w[2]),
        "wq2": wflat(attn2_w[0]), "wk2": wflat(attn2_w[1]),
        "wv2": wflat(attn2_w[2]),
        "w1": np.ascontiguousarray(ffn_w1, dtype=f),
        "w2": np.ascontiguousarray(ffn_w2, dtype=f),
        "bq1": bpart(attn1_b[0]), "bk1": bpart(attn1_b[1]),
        "bv1": np.ascontiguousarray(
            np.asarray(attn1_b[2]).reshape(1, H * DH), dtype=f),
        "bq2": bpart(attn2_b[0]), "bk2": bpart(attn2_b[1]),
        "bv2": np.ascontiguousarray(
            np.asarray(attn2_b[2]).reshape(1, H * DH), dtype=f),
        "b1": np.ascontiguousarray(
            np.asarray(ffn_b1).reshape(NFC, NP).T, dtype=f),
        "b2": np.ascontiguousarray(
            np.asarray(ffn_b2).reshape(NKC, NP).T, dtype=f),
    }
    # Combined diagonal mask patterns for k-chunk pairs: pattern j covers
    # offsets (256j, 256j+128): m01[j][pk, 0:512]   = 1 - mask[q, 256j+pk]
    #                           m01[j][pk, 512:1024] = 1 - mask[q, 256j+128+pk]
    m = np.asarray(mask)
    m01 = np.empty((2, NP, S), f)
    for j in range(2):
        m01[j, :, 0:QB] = 1.0 - m[0:QB, 256 * j:256 * j + NP].T
        m01[j, :, QB:S] = 1.0 - m[0:QB, 256 * j + 128:256 * j + 128 + NP].T
    shared["m01"] = m01

    in_maps = []
    for b in range(B):
        d = dict(shared)
        d["xdeT"] = np.ascontiguousarray(np.asarray(de_x[b]).T, dtype=f)
        d["xenT"] = np.ascontiguousarray(np.asarray(en_x[b]).T, dtype=f)
        in_maps.append(d)
    return in_maps


def get_nc():
    if "nc" not in _CACHE:
        _CACHE["nc"] = _build()
    return _CACHE["nc"]


def kernel(**inputs):
    from concourse.bass_utils import run_bass_kernel_spmd

    nc = get_nc()
    in_maps = _prep_inputs(**inputs)
    res = run_bass_kernel_spmd(nc, in_maps, core_ids=list(range(B)))
    out = np.stack(
        [np.ascontiguousarray(res.results[b]["yT"].T) for b in range(B)], axis=0
    )
    return out.astype(np.float32)

